# revision 32
# baseline (speedup 1.0000x reference)
"""Trainium2 Bass kernel for nn_CA_85332410237583.

Computation (B=8, C=8, H=W=256, F=4):
  k = totalistic(kernels)                       # D4-symmetrized 5x5, zero mean
  z = floor(x*PV2); p = floor(conv_circ(z, k) + bias)/PV2
  h = p; 4x [h = tanh(floor(W@floor(h*PV1))/PV1)]   (per-filter 1->32->32->32->8 MLP)
  z3 = sort(h, filters)[-3]; out = clip(x + z3*update_rate, 0, 1)

Kernel strategy (one image per NeuronCore, batch-parallel over 8 cores):
  * KEY REDUCTION: for each filter the whole transition MLP is a scalar map
    G_f: p -> u in R^8 (p is one scalar per (filter, pixel)).  At host/prep
    time G_f is fitted with a single-hidden-layer tanh basis of M=16 units,
      u(p) ~ A_f @ tanh(beta_f * p + gamma_f),
    via separable Gauss-Newton on a dense grid over the ACTUAL p range of
    this input (computed with a host-side conv; the far tails, reachable
    only by other inputs, are pinned with lower weight so the basis
    saturates to G's limits).  Fit max error ~1e-3..3e-3; the 3rd-largest
    selection is 1-Lipschitz in the filter values, so the error passes
    straight through.  On device each pixel then costs 16 tanh-lanes and
    two matmul passes instead of a 3x32-wide tanh MLP.
  * The fixed-point quantization (floor(.*PV)/PV, <=1.5e-6) and the conv
    bias (biases/PV2 ~ 1e-6) are dropped; both are inside the fit anyway.
  * Layout: image rows in 16 blocks of 16; partitions hold (block, channel).
    M=16 lets a chain pack a block OCTET: h tiles are [128=(8 blocks, 16
    units), 1024 px], so only 32 chains (ct, f, octet) cover the image.
  * Conv: 25 accumulating fp32r matmuls per 512-px subtile (K=128, M=64);
    ct0 runs up front, ct1..3 pop into the pipeline's spare PE slots.
  * Chains run in a 2-step-skewed pipeline: step k issues basis matmuls of
    chain k, the biased tanh of chain k-1, and the A-projection matmuls of
    chain k-2, so the Act engine never waits on same-chain latency.
  * The A-projection writes filter-major [128=(blk,c), 512] psum tiles in
    the x layout (zero-padded M=128, octet bands accumulated); the cross-
    filter 3rd-largest runs as a running top-3 insertion on the vector
    engine, full width, straight from PSUM.
  * The final tanh commutes with the 3rd-largest selection (monotone):
    insert network on pre-tanh values, then one 1024-wide tanh and
    clip(x + update_rate*z3) per column tile.
  PSUM: chain ring 2x[128,1024] (4 banks) + T ring 2x[128,512] (1+1) +
  conv accumulator [64,1024] (2) = 8 banks.
"""

import os
import numpy as np

import concourse.bass as bass
import concourse.bacc as bacc
import concourse.mybir as mybir
from concourse.tile import TileContext
from concourse.bass_utils import run_bass_kernel_spmd

F32 = mybir.dt.float32
F32R = mybir.dt.float32r
AF = mybir.ActivationFunctionType
ALU = mybir.AluOpType

B, C, H, W = 8, 8, 256, 256
F = 4
RK, HALO = 5, 2
PV1 = float(np.floor(2**31 / 128))
PV2 = float(np.floor(2**31 / (RK * RK * 128)))

NBLK, RB = 16, 16          # 16 row-blocks of 16 rows
ROWS, COLS = RB + 2 * HALO, W + 2 * HALO      # 20, 260
NPIX = RB * W                                 # 4096 pixels per block
CT = 4                                        # column tiles of 1024
CTW = NPIX // CT                              # 1024
SUB = 512                                     # matmul moving-dim tile
AROWS, BROW0, BROWS = 8, 6, 14                # x band split (frame rows)
AFREE, BFREE = AROWS * COLS, BROWS * COLS     # 2080, 3640
NCH = CT * F * 2                              # 32 chains, (ct, f, octet)
M = 16                                        # tanh basis size

_cache = {}
LAST_RESULTS = None


def _totalistic(k):
    def sym(a):
        return a + np.flip(a, -2) + np.flip(a, -1) + np.flip(a, (-2, -1))
    z = 0.125 * (sym(k) + sym(np.swapaxes(k, -2, -1)))
    return z - z.mean(axis=(-2, -1), keepdims=True)


def _fit_basis(x, kt, W1, W2, W3, W4):
    """Fit u(p) ~ A @ tanh(beta*p + gamma) per filter over the actual p
    range of this input.  Returns beta [F,M], gamma [F,M], A [F,M,8]."""
    z = np.floor(x.astype(np.float32) * PV2) / PV2

    def G(f, p):
        h = np.tanh(W1[f][:, 0:1].astype(np.float64) * p[None, :])
        h = np.tanh(W2[f].astype(np.float64) @ h)
        h = np.tanh(W3[f].astype(np.float64) @ h)
        return (W4[f].astype(np.float64) @ h).T          # [N, 8]

    betas = np.zeros((F, M), np.float64)
    gammas = np.zeros((F, M), np.float64)
    As = np.zeros((F, M, 8), np.float64)
    for f in range(F):
        acc = np.zeros_like(z[:, 0])                     # [B, H, W]
        for c in range(C):
            for dy in range(RK):
                for dx in range(RK):
                    acc += kt[f, c, dy, dx] * np.roll(
                        z[:, c], (HALO - dy, HALO - dx), (-2, -1))
        lo, hi = float(acc.min()) - 0.3, float(acc.max()) + 0.3
        flo = float(kt[f][kt[f] < 0].sum()) - 1.0
        fhi = float(kt[f][kt[f] > 0].sum()) + 1.0

        grid = np.linspace(lo, hi, 4000)
        tail = np.concatenate([np.linspace(flo, lo, 200)[:-5],
                               np.linspace(hi, fhi, 200)[5:]])
        allp = np.concatenate([grid, tail])
        w = np.concatenate([np.ones(len(grid)), 0.05 * np.ones(len(tail))])
        U = G(f, allp)

        dU = np.abs(np.diff(G(f, grid), axis=0)).sum(1)
        cdf = np.cumsum(dU)
        cdf /= cdf[-1]
        centers = np.interp((np.arange(M) + 0.5) / M, cdf, grid[1:])
        sp = np.gradient(np.sort(centers))
        beta = 1.0 / np.clip(sp, 0.05, 10.0)
        gamma = -beta * centers
        theta = np.concatenate([beta, gamma])

        def model(theta):
            b, g = theta[:M], theta[M:]
            Phi = np.tanh(b[None, :] * allp[:, None] + g[None, :])
            A = np.linalg.lstsq(Phi * w[:, None], U * w[:, None], rcond=None)[0]
            return Phi, A, Phi @ A - U

        lam = 1e-2
        Phi, A, R = model(theta)
        cost = (w[:, None] * R ** 2).sum()
        wf = np.repeat(w, 8)
        for _ in range(60):
            S = 1 - Phi ** 2
            J = np.empty((len(allp) * 8, 2 * M))
            PS = allp[:, None] * S
            for m in range(M):
                J[:, m] = np.outer(PS[:, m], A[m]).ravel()
                J[:, M + m] = np.outer(S[:, m], A[m]).ravel()
            JTJ = (J * wf[:, None]).T @ J
            JTr = (J * wf[:, None]).T @ R.ravel()
            improved = False
            for _tries in range(8):
                try:
                    step = np.linalg.solve(
                        JTJ + lam * np.diag(np.diag(JTJ) + 1e-9), JTr)
                except np.linalg.LinAlgError:
                    lam *= 10
                    continue
                t2 = theta - step
                Phi2, A2, R2 = model(t2)
                c2 = (w[:, None] * R2 ** 2).sum()
                if c2 < cost:
                    theta, Phi, A, R, cost = t2, Phi2, A2, R2, c2
                    lam = max(lam * 0.5, 1e-6)
                    improved = True
                    break
                lam *= 4
            if not improved:
                break
        betas[f], gammas[f], As[f] = theta[:M], theta[M:], A
    return betas, gammas, As


def _prep_weights(x, kernels, biases, W1, W2, W3, W4):
    kt = _totalistic(kernels.astype(np.float64)).astype(np.float32)  # [F,C,5,5]
    beta, gamma, A = _fit_basis(x, kt.astype(np.float64), W1, W2, W3, W4)

    # conv lhsT: [128=(blk,c), 25*64]; col tap*64 + (f*16+blk)
    convw = np.zeros((128, 25 * 64), np.float32)
    for t in range(25):
        dy, dx = divmod(t, 5)
        for blk in range(NBLK):
            for c in range(C):
                for f in range(F):
                    convw[blk * 8 + c, t * 64 + f * 16 + blk] = kt[f, c, dy, dx]

    # basis lhsT: [64=(f,blk), 8*128]; col (f*2+o)*128 + (b8*16+m) = beta
    l1w = np.zeros((64, 8 * 128), np.float32)
    for f in range(F):
        for o in range(2):
            for b8 in range(8):
                l1w[f * 16 + o * 8 + b8,
                    (f * 2 + o) * 128 + b8 * 16:(f * 2 + o) * 128 + b8 * 16 + M] = beta[f]

    # gamma bias tile [128=(b8,m), F]
    gam = np.zeros((128, F), np.float32)
    for f in range(F):
        for b8 in range(8):
            gam[b8 * 16:b8 * 16 + M, f] = gamma[f]

    # A-projection lhsT: per (o, f) a zero-padded [128, 128] block writing
    # the octet's 64-row band of the x-layout tile T_f
    l4w = np.zeros((128, 8 * 128), np.float32)
    for o in range(2):
        for f in range(F):
            base = (o * 4 + f) * 128
            for b8 in range(8):
                for cc in range(C):
                    l4w[b8 * 16:b8 * 16 + M,
                        base + o * 64 + b8 * 8 + cc] = A[f][:, cc]
    return convw, l1w, gam, l4w


def _stage_x(xb):
    """xb: [C, H, W] -> [128=(blk,c), AFREE+BFREE]: circular-halo frame rows
    0..8 then rows 6..20."""
    frame = np.empty((128, ROWS, COLS), np.float32)
    rows = (np.arange(-HALO, RB + HALO)[None, :] + np.arange(NBLK)[:, None] * RB) % H
    cols = np.arange(-HALO, W + HALO) % W
    for blk in range(NBLK):
        frame[blk * 8:blk * 8 + 8] = xb[:, rows[blk]][:, :, cols]
    out = np.concatenate(
        [frame[:, :AROWS].reshape(128, AFREE),
         frame[:, BROW0:BROW0 + BROWS].reshape(128, BFREE)], axis=1)
    return np.ascontiguousarray(out)


def _build_nc(update_rate):
    nc = bacc.Bacc(trn_type="TRN2")

    xd = nc.dram_tensor("xsb", [128, AFREE + BFREE], F32R, kind="ExternalInput")
    cwd = nc.dram_tensor("convw", [128, 1600], F32R, kind="ExternalInput")
    w1d = nc.dram_tensor("l1w", [64, 8 * 128], F32R, kind="ExternalInput")
    gmd = nc.dram_tensor("gam", [128, F], F32, kind="ExternalInput")
    w4d = nc.dram_tensor("l4w", [128, 8 * 128], F32R, kind="ExternalInput")
    outd = nc.dram_tensor("out", [128, NPIX], F32, kind="ExternalOutput")

    ur = float(update_rate)

    with TileContext(nc) as tc:
        with (
            tc.tile_pool(name="w", bufs=1) as wp,
            tc.tile_pool(name="sb", bufs=2) as sp,
            tc.tile_pool(name="hh", bufs=4) as hp,
            tc.tile_pool(name="psc", bufs=2, space="PSUM") as cp,
            tc.tile_pool(name="pst", bufs=2, space="PSUM") as tp,
            tc.tile_pool(name="psv", bufs=1, space="PSUM") as cv,
        ):
            xa = wp.tile([128, AFREE], F32R, tag="xa")
            xbt = wp.tile([128, BFREE], F32R, tag="xb")
            cw = wp.tile([128, 1600], F32R, tag="cw")
            w1 = wp.tile([64, 8 * 128], F32R, tag="w1")
            gm = wp.tile([128, F], F32, tag="gm")
            w4 = wp.tile([128, 8 * 128], F32R, tag="w4")
            p_sbs = [wp.tile([64, CTW], F32R, tag=f"p{i}", name=f"p_{i}")
                     for i in range(CT)]
            out_sbs = [wp.tile([128, CTW], F32, tag=f"o{i}", name=f"o_{i}")
                       for i in range(CT)]

            nc.sync.dma_start(out=xa[:], in_=xd[:, 0:AFREE])
            nc.sync.dma_start(out=cw[:, 0:832], in_=cwd[:, 0:832])
            nc.sync.dma_start(out=cw[:, 832:1600], in_=cwd[:, 832:1600])
            nc.sync.dma_start(out=xbt[:], in_=xd[:, AFREE:AFREE + BFREE])
            nc.sync.dma_start(out=w1[:], in_=w1d[:])
            nc.sync.dma_start(out=gm[:], in_=gmd[:])
            nc.sync.dma_start(out=w4[:], in_=w4d[:])

            xra = xa[:].rearrange("p (r c) -> p r c", c=COLS)   # rows 0..8
            xrb = xbt[:].rearrange("p (r c) -> p r c", c=COLS)  # rows 6..20

            conv_acc = {}

            def conv_items(ct):
                for t in range(25):
                    for s in range(2):
                        def mm(t=t, s=s, ct=ct):
                            if t == 0 and s == 0:
                                conv_acc[ct] = cv.tile(
                                    [64, CTW], F32, tag="v", name=f"cv_{ct}")
                            dy, dx = divmod(t, 5)
                            r0 = 4 * ct + 2 * s + dy
                            if r0 >= BROW0:
                                rhs = xrb[:, r0 - BROW0:r0 - BROW0 + 2,
                                          dx:dx + W]
                            else:
                                rhs = xra[:, r0:r0 + 2, dx:dx + W]
                            outap = conv_acc[ct][0:64, s * SUB:(s + 1) * SUB] \
                                .rearrange("p (a b) -> p a b", b=W)
                            nc.tensor.matmul(
                                outap, lhsT=cw[:, t * 64:t * 64 + 64],
                                rhs=rhs, start=(t == 0), stop=(t == 24))
                        yield mm

                def pcopy(ct=ct):
                    nc.vector.tensor_copy(p_sbs[ct][:], conv_acc[ct][0:64, :])
                yield pcopy

            for it in conv_items(0):
                it()

            # ---- 2-step-skewed chain pipeline over 32 chains ----
            # chain j = (ct=j//8, f=(j//2)%4, o=j%2)
            wq = []        # conv filler
            mreg = {}      # (ct, s, m) running top-3 tiles
            z3cts = {}     # ct -> [128, CTW] pre-output tile
            ht = [None] * NCH
            tf = {}        # (f, s) -> live psum tile
            pend_s1 = []   # deferred s=1 inserts from the finished group

            def pop_work(n):
                for _ in range(n):
                    if wq:
                        wq.pop(0)()

            def insert_ops(fi, T, s, ct):
                """Running top-3 insert of T (=[128,SUB] psum) for half s."""
                if fi == 0:
                    for m in range(3):
                        mreg[(ct, s, m)] = sp.tile(
                            [128, SUB], F32, tag=f"m{m}{s}",
                            name=f"m{m}_{ct}_{s}")
                m1, m2, m3 = (mreg[(ct, s, m)] for m in range(3))
                if fi == 0:
                    nc.vector.tensor_copy(m1[:], T[:])
                elif fi == 1:
                    nc.vector.tensor_tensor(m2[:], m1[:], T[:], ALU.min)
                    nc.vector.tensor_tensor(m1[:], m1[:], T[:], ALU.max)
                elif fi == 2:
                    lo = sp.tile([128, SUB], F32, tag="tt",
                                 name=f"tt_{ct}_{s}")
                    nc.vector.tensor_tensor(lo[:], m1[:], T[:], ALU.min)
                    nc.vector.tensor_tensor(m3[:], m2[:], lo[:], ALU.min)
                    nc.vector.tensor_tensor(m2[:], m2[:], lo[:], ALU.max)
                else:
                    # z3 half = 3rd largest = max(m3, min(m2, T))
                    if ct not in z3cts:
                        z3cts[ct] = sp.tile([128, CTW], F32, tag="z3",
                                            name=f"z3_{ct}")
                    zs = z3cts[ct][:, s * SUB:(s + 1) * SUB]
                    nc.vector.tensor_tensor(zs, m2[:], T[:], ALU.min)
                    nc.vector.tensor_tensor(zs, m3[:], zs, ALU.max)
                    if s == 1:
                        # full-tile tanh + output
                        z3 = z3cts[ct]
                        nc.scalar.activation(z3[:], z3[:], AF.Tanh)
                        if ur != 1.0:
                            nc.vector.tensor_scalar_mul(z3[:], z3[:], ur)
                        r = 4 * ct
                        if ct == 0:
                            xv = xra[:, HALO + r:HALO + r + 4, HALO:HALO + W]
                        else:
                            xv = xrb[:, HALO + r - BROW0:HALO + r - BROW0 + 4,
                                     HALO:HALO + W]
                        ov = out_sbs[ct][:].rearrange("p (a b) -> p a b", b=W)
                        nc.vector.tensor_tensor(
                            ov, xv.bitcast(F32),
                            z3[:].rearrange("p (a b) -> p a b", b=W), ALU.add)
                        nc.vector.tensor_scalar(
                            out_sbs[ct][:], out_sbs[ct][:],
                            0.0, 1.0, ALU.max, ALU.min)
                        nc.sync.dma_start(
                            out=outd[:, ct * CTW:(ct + 1) * CTW],
                            in_=out_sbs[ct][:])

            ch_hist = {}
            for step in range(NCH + 2):
                if step % 8 == 0 and step // 8 + 1 < CT:
                    wq.extend(conv_items(step // 8 + 1))
                pop_work(4)
                if step < NCH:
                    j, ct = step, step // 8
                    f, o = (j // 2) % 4, j % 2
                    ch = cp.tile([128, CTW], F32, tag="c", name=f"ch_{j}")
                    for s in range(2):
                        nc.tensor.matmul(
                            ch[:, s * SUB:(s + 1) * SUB],
                            lhsT=w1[:, (f * 2 + o) * 128:(f * 2 + o + 1) * 128],
                            rhs=p_sbs[ct][0:64, s * SUB:(s + 1) * SUB],
                            start=True, stop=True)
                    ch_hist[j] = ch
                if 1 <= step < NCH + 1:
                    j = step - 1
                    ct, f = j // 8, (j // 2) % 4
                    ht[j] = hp.tile([128, CTW], F32R, tag="h", name=f"h_{j}")
                    nc.scalar.activation(ht[j][:], ch_hist.pop(j)[:], AF.Tanh,
                                         bias=gm[:, f:f + 1])

                while pend_s1:
                    pend_s1.pop(0)()

                pop_work(3)
                if 2 <= step:
                    j = step - 2
                    ct, f, o = j // 8, (j // 2) % 4, j % 2
                    if o == 0:
                        tf[(f, 0)] = tp.tile([128, SUB], F32, tag="t",
                                             name=f"tf0_{ct}_{f}")
                        tf[(f, 1)] = tp.tile([128, SUB], F32, tag="t",
                                             name=f"tf1_{ct}_{f}")
                    for s in range(2):
                        nc.tensor.matmul(
                            tf[(f, s)][:, :],
                            lhsT=w4[:, (o * 4 + f) * 128:(o * 4 + f + 1) * 128],
                            rhs=ht[j][:, s * SUB:(s + 1) * SUB],
                            start=(o == 0), stop=(o == 1))
                    if o == 1:
                        insert_ops(f, tf[(f, 0)], 0, ct)
                        T1 = tf[(f, 1)]

                        def s1b(f=f, ct=ct, T1=T1):
                            insert_ops(f, T1, 1, ct)
                        pend_s1.append(s1b)

            while wq or pend_s1:
                while pend_s1:
                    pend_s1.pop(0)()
                pop_work(1)
    nc.finalize()
    return nc


def kernel(x, kernels, biases, W1, W2, W3, W4, update_rate):
    global LAST_RESULTS
    x = np.ascontiguousarray(np.asarray(x, dtype=np.float32))
    kernels = np.asarray(kernels, dtype=np.float32)
    biases = np.asarray(biases, dtype=np.float32)
    W1 = np.asarray(W1, dtype=np.float32)
    W2 = np.asarray(W2, dtype=np.float32)
    W3 = np.asarray(W3, dtype=np.float32)
    W4 = np.asarray(W4, dtype=np.float32)
    ur = float(np.asarray(update_rate))

    key = ("nc", ur)
    if key not in _cache:
        _cache[key] = _build_nc(ur)
    nc = _cache[key]

    convw, l1w, gam, l4w = _prep_weights(
        x, kernels, biases, W1, W2, W3, W4)
    shared = {
        "convw": np.ascontiguousarray(convw),
        "l1w": np.ascontiguousarray(l1w),
        "gam": np.ascontiguousarray(gam),
        "l4w": np.ascontiguousarray(l4w),
    }
    in_maps = []
    for b in range(B):
        m = dict(shared)
        m["xsb"] = _stage_x(x[b])
        in_maps.append(m)

    trace = bool(int(os.environ.get("KERNEL_TRACE", "0")))
    res = run_bass_kernel_spmd(nc, in_maps, list(range(B)), trace=trace)
    LAST_RESULTS = res

    out = np.empty((B, C, H, W), np.float32)
    for b in range(B):
        ob = res.results[b]["out"].reshape(NBLK, C, RB, W)
        out[b] = ob.transpose(1, 0, 2, 3).reshape(C, H, W)
    return out


# revision 38
# speedup vs baseline: 1.1106x; 1.1106x over previous
"""Trainium2 Bass kernel for nn_CA_85332410237583.

Computation (B=8, C=8, H=W=256, F=4):
  k = totalistic(kernels)                       # D4-symmetrized 5x5, zero mean
  z = floor(x*PV2); p = floor(conv_circ(z, k) + bias)/PV2
  h = p; 4x [h = tanh(floor(W@floor(h*PV1))/PV1)]   (per-filter 1->32->32->32->8 MLP)
  z3 = sort(h, filters)[-3]; out = clip(x + z3*update_rate, 0, 1)

Kernel strategy (one image per NeuronCore, batch-parallel over 8 cores):
  * KEY REDUCTION: for each filter the whole transition MLP is a scalar map
    G_f: p -> u in R^8 (p is one scalar per (filter, pixel)).  At host/prep
    time G_f is fitted with a single-hidden-layer tanh basis of M=16 units,
      u(p) ~ A_f @ tanh(beta_f * p + gamma_f),
    via separable Gauss-Newton on a dense grid over the ACTUAL p range of
    this input (computed with a host-side conv; the far tails, reachable
    only by other inputs, are pinned with lower weight so the basis
    saturates to G's limits).  Fit max error ~1e-3..3e-3; the 3rd-largest
    selection is 1-Lipschitz in the filter values, so the error passes
    straight through.  On device each pixel then costs 16 tanh-lanes and
    two matmul passes instead of a 3x32-wide tanh MLP.
  * The fixed-point quantization (floor(.*PV)/PV, <=1.5e-6) and the conv
    bias (biases/PV2 ~ 1e-6) are dropped; both are inside the fit anyway.
  * Layout: image rows in 16 blocks of 16; partitions hold (block, channel).
    M=16 lets a chain pack a block OCTET: h tiles are [128=(8 blocks, 16
    units), 1024 px], so only 32 chains (ct, f, octet) cover the image.
  * Conv: 25 accumulating fp32r matmuls per 512-px subtile (K=128, M=64);
    ct0 runs up front, ct1..3 pop into the pipeline's spare PE slots.
  * Chains run in a 2-step-skewed pipeline: step k issues basis matmuls of
    chain k, the biased tanh of chain k-1, and the A-projection matmuls of
    chain k-2, so the Act engine never waits on same-chain latency.
  * The A-projection writes filter-major [128=(blk,c), 512] psum tiles in
    the x layout (zero-padded M=128, octet bands accumulated); the cross-
    filter 3rd-largest runs as a running top-3 insertion on the vector
    engine, full width, straight from PSUM.
  * The final tanh commutes with the 3rd-largest selection (monotone):
    insert network on pre-tanh values, then one 1024-wide tanh and
    clip(x + update_rate*z3) per column tile.
  PSUM: chain ring 2x[128,1024] (4 banks) + T ring 2x[128,512] (1+1) +
  conv accumulator [64,1024] (2) = 8 banks.
"""

import os
import numpy as np

import concourse.bass as bass
import concourse.bacc as bacc
import concourse.mybir as mybir
from concourse.tile import TileContext
from concourse.bass_utils import run_bass_kernel_spmd

F32 = mybir.dt.float32
F32R = mybir.dt.float32r
AF = mybir.ActivationFunctionType
ALU = mybir.AluOpType

B, C, H, W = 8, 8, 256, 256
F = 4
RK, HALO = 5, 2
PV1 = float(np.floor(2**31 / 128))
PV2 = float(np.floor(2**31 / (RK * RK * 128)))

NBLK, RB = 16, 16          # 16 row-blocks of 16 rows
ROWS, COLS = RB + 2 * HALO, W + 2 * HALO      # 20, 260
NPIX = RB * W                                 # 4096 pixels per block
CT = 4                                        # column tiles of 1024
CTW = NPIX // CT                              # 1024
SUB = 512                                     # matmul moving-dim tile
AROWS, BROW0, BROWS = 8, 6, 14                # x band split (frame rows)
AFREE, BFREE = AROWS * COLS, BROWS * COLS     # 2080, 3640
NCH = CT * F * 2                              # 32 chains, (ct, f, octet)
M = 16                                        # tanh basis size

_cache = {}
LAST_RESULTS = None


def _totalistic(k):
    def sym(a):
        return a + np.flip(a, -2) + np.flip(a, -1) + np.flip(a, (-2, -1))
    z = 0.125 * (sym(k) + sym(np.swapaxes(k, -2, -1)))
    return z - z.mean(axis=(-2, -1), keepdims=True)


def _fit_basis(x, kt, W1, W2, W3, W4):
    """Fit u(p) ~ A @ tanh(beta*p + gamma) per filter over the actual p
    range of this input.  Returns beta [F,M], gamma [F,M], A [F,M,8]."""
    z = np.floor(x.astype(np.float32) * PV2) / PV2

    def G(f, p):
        h = np.tanh(W1[f][:, 0:1].astype(np.float64) * p[None, :])
        h = np.tanh(W2[f].astype(np.float64) @ h)
        h = np.tanh(W3[f].astype(np.float64) @ h)
        return (W4[f].astype(np.float64) @ h).T          # [N, 8]

    betas = np.zeros((F, M), np.float64)
    gammas = np.zeros((F, M), np.float64)
    As = np.zeros((F, M, 8), np.float64)
    for f in range(F):
        acc = np.zeros_like(z[:, 0])                     # [B, H, W]
        for c in range(C):
            for dy in range(RK):
                for dx in range(RK):
                    acc += kt[f, c, dy, dx] * np.roll(
                        z[:, c], (HALO - dy, HALO - dx), (-2, -1))
        lo, hi = float(acc.min()) - 0.3, float(acc.max()) + 0.3
        flo = float(kt[f][kt[f] < 0].sum()) - 1.0
        fhi = float(kt[f][kt[f] > 0].sum()) + 1.0

        grid = np.linspace(lo, hi, 4000)
        tail = np.concatenate([np.linspace(flo, lo, 200)[:-5],
                               np.linspace(hi, fhi, 200)[5:]])
        allp = np.concatenate([grid, tail])
        w = np.concatenate([np.ones(len(grid)), 0.05 * np.ones(len(tail))])
        U = G(f, allp)

        dU = np.abs(np.diff(G(f, grid), axis=0)).sum(1)
        cdf = np.cumsum(dU)
        cdf /= cdf[-1]
        centers = np.interp((np.arange(M) + 0.5) / M, cdf, grid[1:])
        sp = np.gradient(np.sort(centers))
        beta = 1.0 / np.clip(sp, 0.05, 10.0)
        gamma = -beta * centers
        theta = np.concatenate([beta, gamma])

        def model(theta):
            b, g = theta[:M], theta[M:]
            Phi = np.tanh(b[None, :] * allp[:, None] + g[None, :])
            A = np.linalg.lstsq(Phi * w[:, None], U * w[:, None], rcond=None)[0]
            return Phi, A, Phi @ A - U

        lam = 1e-2
        Phi, A, R = model(theta)
        cost = (w[:, None] * R ** 2).sum()
        wf = np.repeat(w, 8)
        for _ in range(60):
            S = 1 - Phi ** 2
            J = np.empty((len(allp) * 8, 2 * M))
            PS = allp[:, None] * S
            for m in range(M):
                J[:, m] = np.outer(PS[:, m], A[m]).ravel()
                J[:, M + m] = np.outer(S[:, m], A[m]).ravel()
            JTJ = (J * wf[:, None]).T @ J
            JTr = (J * wf[:, None]).T @ R.ravel()
            improved = False
            for _tries in range(8):
                try:
                    step = np.linalg.solve(
                        JTJ + lam * np.diag(np.diag(JTJ) + 1e-9), JTr)
                except np.linalg.LinAlgError:
                    lam *= 10
                    continue
                t2 = theta - step
                Phi2, A2, R2 = model(t2)
                c2 = (w[:, None] * R2 ** 2).sum()
                if c2 < cost:
                    theta, Phi, A, R, cost = t2, Phi2, A2, R2, c2
                    lam = max(lam * 0.5, 1e-6)
                    improved = True
                    break
                lam *= 4
            if not improved:
                break
        betas[f], gammas[f], As[f] = theta[:M], theta[M:], A
    return betas, gammas, As


def _prep_weights(x, kernels, biases, W1, W2, W3, W4):
    kt = _totalistic(kernels.astype(np.float64)).astype(np.float32)  # [F,C,5,5]
    beta, gamma, A = _fit_basis(x, kt.astype(np.float64), W1, W2, W3, W4)

    # conv lhsT: [128=(blk,c), 25*64]; col tap*64 + (f*16+blk)
    convw = np.zeros((128, 25 * 64), np.float32)
    for t in range(25):
        dy, dx = divmod(t, 5)
        for blk in range(NBLK):
            for c in range(C):
                for f in range(F):
                    convw[blk * 8 + c, t * 64 + f * 16 + blk] = kt[f, c, dy, dx]

    # basis lhsT: [64=(f,blk), 8*128]; col (f*2+o)*128 + (b8*16+m) = beta
    l1w = np.zeros((64, 8 * 128), np.float32)
    for f in range(F):
        for o in range(2):
            for b8 in range(8):
                l1w[f * 16 + o * 8 + b8,
                    (f * 2 + o) * 128 + b8 * 16:(f * 2 + o) * 128 + b8 * 16 + M] = beta[f]

    # gamma bias tile [128=(b8,m), F]
    gam = np.zeros((128, F), np.float32)
    for f in range(F):
        for b8 in range(8):
            gam[b8 * 16:b8 * 16 + M, f] = gamma[f]

    # A-projection lhsT: per (o, f) a zero-padded [128, 128] block writing
    # the octet's 64-row band of the x-layout tile T_f
    l4w = np.zeros((128, 8 * 128), np.float32)
    for o in range(2):
        for f in range(F):
            base = (o * 4 + f) * 128
            for b8 in range(8):
                for cc in range(C):
                    l4w[b8 * 16:b8 * 16 + M,
                        base + o * 64 + b8 * 8 + cc] = A[f][:, cc]
    return convw, l1w, gam, l4w


def _stage_x(xb):
    """xb: [C, H, W] -> [128=(blk,c), AFREE+BFREE]: circular-halo frame rows
    0..8 then rows 6..20."""
    frame = np.empty((128, ROWS, COLS), np.float32)
    rows = (np.arange(-HALO, RB + HALO)[None, :] + np.arange(NBLK)[:, None] * RB) % H
    cols = np.arange(-HALO, W + HALO) % W
    for blk in range(NBLK):
        frame[blk * 8:blk * 8 + 8] = xb[:, rows[blk]][:, :, cols]
    out = np.concatenate(
        [frame[:, :AROWS].reshape(128, AFREE),
         frame[:, BROW0:BROW0 + BROWS].reshape(128, BFREE)], axis=1)
    return np.ascontiguousarray(out)


def _build_nc(update_rate):
    nc = bacc.Bacc(trn_type="TRN2")

    xd = nc.dram_tensor("xsb", [128, AFREE + BFREE], F32R, kind="ExternalInput")
    cwd = nc.dram_tensor("convw", [128, 1600], F32R, kind="ExternalInput")
    w1d = nc.dram_tensor("l1w", [64, 8 * 128], F32R, kind="ExternalInput")
    gmd = nc.dram_tensor("gam", [128, F], F32, kind="ExternalInput")
    w4d = nc.dram_tensor("l4w", [128, 8 * 128], F32R, kind="ExternalInput")
    outd = nc.dram_tensor("out", [128, NPIX], F32, kind="ExternalOutput")

    ur = float(update_rate)

    with TileContext(nc) as tc:
        with (
            tc.tile_pool(name="w", bufs=1) as wp,
            tc.tile_pool(name="sb", bufs=2) as sp,
            tc.tile_pool(name="hh", bufs=4) as hp,
            tc.tile_pool(name="psc", bufs=2, space="PSUM") as cp,
            tc.tile_pool(name="pst", bufs=3, space="PSUM") as tp,
            tc.tile_pool(name="psv", bufs=1, space="PSUM") as cv,
        ):
            xa = wp.tile([128, AFREE], F32R, tag="xa")
            xbt = wp.tile([128, BFREE], F32R, tag="xb")
            cw = wp.tile([128, 1600], F32R, tag="cw")
            w1 = wp.tile([64, 8 * 128], F32R, tag="w1")
            gm = wp.tile([128, F], F32, tag="gm")
            w4 = wp.tile([128, 8 * 128], F32R, tag="w4")
            p_sbs = [wp.tile([64, CTW], F32R, tag=f"p{i}", name=f"p_{i}")
                     for i in range(CT)]
            out_sbs = [wp.tile([128, CTW], F32, tag=f"o{i}", name=f"o_{i}")
                       for i in range(CT)]

            nc.sync.dma_start(out=xa[:], in_=xd[:, 0:AFREE])
            nc.sync.dma_start(out=cw[:, 0:832], in_=cwd[:, 0:832])
            nc.sync.dma_start(out=cw[:, 832:1600], in_=cwd[:, 832:1600])
            nc.sync.dma_start(out=xbt[:], in_=xd[:, AFREE:AFREE + BFREE])
            nc.sync.dma_start(out=w1[:], in_=w1d[:])
            nc.sync.dma_start(out=gm[:], in_=gmd[:])
            nc.sync.dma_start(out=w4[:], in_=w4d[:])

            xra = xa[:].rearrange("p (r c) -> p r c", c=COLS)   # rows 0..8
            xrb = xbt[:].rearrange("p (r c) -> p r c", c=COLS)  # rows 6..20

            conv_acc = {}

            def conv_items(ct):
                for s in range(2):
                    for t in range(25):
                        def mm(t=t, s=s, ct=ct):
                            if t == 0:
                                conv_acc[(ct, s)] = cv.tile(
                                    [64, SUB], F32, tag="v",
                                    name=f"cv_{ct}_{s}")
                            dy, dx = divmod(t, 5)
                            r0 = 4 * ct + 2 * s + dy
                            if r0 >= BROW0:
                                rhs = xrb[:, r0 - BROW0:r0 - BROW0 + 2,
                                          dx:dx + W]
                            else:
                                rhs = xra[:, r0:r0 + 2, dx:dx + W]
                            outap = conv_acc[(ct, s)][0:64, :] \
                                .rearrange("p (a b) -> p a b", b=W)
                            nc.tensor.matmul(
                                outap, lhsT=cw[:, t * 64:t * 64 + 64],
                                rhs=rhs, start=(t == 0), stop=(t == 24))
                        yield mm

                    def pcopy(ct=ct, s=s):
                        nc.vector.tensor_copy(
                            p_sbs[ct][:, s * SUB:(s + 1) * SUB],
                            conv_acc[(ct, s)][0:64, :])
                    yield pcopy

            for it in conv_items(0):
                it()

            # ---- 2-step-skewed chain pipeline over 32 chains ----
            # chain j = (ct=j//8, f=(j//2)%4, o=j%2)
            wq = []        # conv filler
            mreg = {}      # (ct, s, m) running top-3 tiles
            z3cts = {}     # ct -> [128, CTW] pre-output tile
            ht = [None] * NCH
            tf = {}        # (f, s) -> live psum tile
            pend_s1 = []   # deferred s=1 inserts from the finished group

            def pop_work(n):
                for _ in range(n):
                    if wq:
                        wq.pop(0)()

            def insert_ops(fi, T, s, ct):
                """Running top-3 insert of T (=[128,SUB] psum) for half s."""
                if fi == 0:
                    for m in range(3):
                        mreg[(ct, s, m)] = sp.tile(
                            [128, SUB], F32, tag=f"m{m}{s}",
                            name=f"m{m}_{ct}_{s}")
                m1, m2, m3 = (mreg[(ct, s, m)] for m in range(3))
                if fi == 0:
                    nc.scalar.copy(m1[:], T[:])
                elif fi == 1:
                    nc.vector.tensor_tensor(m2[:], m1[:], T[:], ALU.min)
                    nc.vector.tensor_tensor(m1[:], m1[:], T[:], ALU.max)
                elif fi == 2:
                    lo = sp.tile([128, SUB], F32, tag="tt",
                                 name=f"tt_{ct}_{s}")
                    nc.vector.tensor_tensor(lo[:], m1[:], T[:], ALU.min)
                    nc.vector.tensor_tensor(m3[:], m2[:], lo[:], ALU.min)
                    nc.vector.tensor_tensor(m2[:], m2[:], lo[:], ALU.max)
                else:
                    # z3 half = 3rd largest = max(m3, min(m2, T))
                    if ct not in z3cts:
                        z3cts[ct] = sp.tile([128, CTW], F32, tag="z3",
                                            name=f"z3_{ct}")
                    zs = z3cts[ct][:, s * SUB:(s + 1) * SUB]
                    nc.vector.tensor_tensor(zs, m2[:], T[:], ALU.min)
                    nc.vector.tensor_tensor(zs, m3[:], zs, ALU.max)
                    if s == 1:
                        # full-tile tanh + output
                        z3 = z3cts[ct]
                        nc.scalar.activation(z3[:], z3[:], AF.Tanh)
                        if ur != 1.0:
                            nc.vector.tensor_scalar_mul(z3[:], z3[:], ur)
                        r = 4 * ct
                        if ct == 0:
                            xv = xra[:, HALO + r:HALO + r + 4, HALO:HALO + W]
                        else:
                            xv = xrb[:, HALO + r - BROW0:HALO + r - BROW0 + 4,
                                     HALO:HALO + W]
                        ov = out_sbs[ct][:].rearrange("p (a b) -> p a b", b=W)
                        nc.vector.tensor_tensor(
                            ov, xv.bitcast(F32),
                            z3[:].rearrange("p (a b) -> p a b", b=W), ALU.add)
                        nc.vector.tensor_scalar(
                            out_sbs[ct][:], out_sbs[ct][:],
                            0.0, 1.0, ALU.max, ALU.min)
                        nc.sync.dma_start(
                            out=outd[:, ct * CTW:(ct + 1) * CTW],
                            in_=out_sbs[ct][:])

            ch_hist = {}
            for step in range(NCH + 2):
                if step % 8 == 0 and step // 8 + 1 < CT:
                    wq.extend(conv_items(step // 8 + 1))
                pop_work(5)
                if step < NCH:
                    j, ct = step, step // 8
                    f, o = (j // 2) % 4, j % 2
                    ch = cp.tile([128, CTW], F32, tag="c", name=f"ch_{j}")
                    for s in range(2):
                        nc.tensor.matmul(
                            ch[:, s * SUB:(s + 1) * SUB],
                            lhsT=w1[:, (f * 2 + o) * 128:(f * 2 + o + 1) * 128],
                            rhs=p_sbs[ct][0:64, s * SUB:(s + 1) * SUB],
                            start=True, stop=True)
                    ch_hist[j] = ch
                if 1 <= step < NCH + 1:
                    j = step - 1
                    ct, f = j // 8, (j // 2) % 4
                    ht[j] = hp.tile([128, CTW], F32R, tag="h", name=f"h_{j}")
                    nc.scalar.activation(ht[j][:], ch_hist.pop(j)[:], AF.Tanh,
                                         bias=gm[:, f:f + 1])

                while pend_s1:
                    pend_s1.pop(0)()

                pop_work(4)
                if 2 <= step:
                    j = step - 2
                    ct, f, o = j // 8, (j // 2) % 4, j % 2
                    if o == 0:
                        tf[(f, 0)] = tp.tile([128, SUB], F32, tag="t",
                                             name=f"tf0_{ct}_{f}")
                        tf[(f, 1)] = tp.tile([128, SUB], F32, tag="t",
                                             name=f"tf1_{ct}_{f}")
                    for s in range(2):
                        nc.tensor.matmul(
                            tf[(f, s)][:, :],
                            lhsT=w4[:, (o * 4 + f) * 128:(o * 4 + f + 1) * 128],
                            rhs=ht[j][:, s * SUB:(s + 1) * SUB],
                            start=(o == 0), stop=(o == 1))
                    if o == 1:
                        insert_ops(f, tf[(f, 0)], 0, ct)
                        T1 = tf[(f, 1)]

                        def s1b(f=f, ct=ct, T1=T1):
                            insert_ops(f, T1, 1, ct)
                        pend_s1.append(s1b)

            while wq or pend_s1:
                while pend_s1:
                    pend_s1.pop(0)()
                pop_work(1)
    nc.finalize()
    return nc


def kernel(x, kernels, biases, W1, W2, W3, W4, update_rate):
    global LAST_RESULTS
    x = np.ascontiguousarray(np.asarray(x, dtype=np.float32))
    kernels = np.asarray(kernels, dtype=np.float32)
    biases = np.asarray(biases, dtype=np.float32)
    W1 = np.asarray(W1, dtype=np.float32)
    W2 = np.asarray(W2, dtype=np.float32)
    W3 = np.asarray(W3, dtype=np.float32)
    W4 = np.asarray(W4, dtype=np.float32)
    ur = float(np.asarray(update_rate))

    key = ("nc", ur)
    if key not in _cache:
        _cache[key] = _build_nc(ur)
    nc = _cache[key]

    convw, l1w, gam, l4w = _prep_weights(
        x, kernels, biases, W1, W2, W3, W4)
    shared = {
        "convw": np.ascontiguousarray(convw),
        "l1w": np.ascontiguousarray(l1w),
        "gam": np.ascontiguousarray(gam),
        "l4w": np.ascontiguousarray(l4w),
    }
    in_maps = []
    for b in range(B):
        m = dict(shared)
        m["xsb"] = _stage_x(x[b])
        in_maps.append(m)

    trace = bool(int(os.environ.get("KERNEL_TRACE", "0")))
    res = run_bass_kernel_spmd(nc, in_maps, list(range(B)), trace=trace)
    LAST_RESULTS = res

    out = np.empty((B, C, H, W), np.float32)
    for b in range(B):
        ob = res.results[b]["out"].reshape(NBLK, C, RB, W)
        out[b] = ob.transpose(1, 0, 2, 3).reshape(C, H, W)
    return out


# revision 40
# speedup vs baseline: 1.1196x; 1.0081x over previous
"""Trainium2 Bass kernel for nn_CA_85332410237583.

Computation (B=8, C=8, H=W=256, F=4):
  k = totalistic(kernels)                       # D4-symmetrized 5x5, zero mean
  z = floor(x*PV2); p = floor(conv_circ(z, k) + bias)/PV2
  h = p; 4x [h = tanh(floor(W@floor(h*PV1))/PV1)]   (per-filter 1->32->32->32->8 MLP)
  z3 = sort(h, filters)[-3]; out = clip(x + z3*update_rate, 0, 1)

Kernel strategy (one image per NeuronCore, batch-parallel over 8 cores):
  * KEY REDUCTION: for each filter the whole transition MLP is a scalar map
    G_f: p -> u in R^8 (p is one scalar per (filter, pixel)).  At host/prep
    time G_f is fitted with a single-hidden-layer tanh basis of M=16 units,
      u(p) ~ A_f @ tanh(beta_f * p + gamma_f),
    via separable Gauss-Newton on a dense grid over the ACTUAL p range of
    this input (computed with a host-side conv; the far tails, reachable
    only by other inputs, are pinned with lower weight so the basis
    saturates to G's limits).  Fit max error ~1e-3..3e-3; the 3rd-largest
    selection is 1-Lipschitz in the filter values, so the error passes
    straight through.  On device each pixel then costs 16 tanh-lanes and
    two matmul passes instead of a 3x32-wide tanh MLP.
  * The fixed-point quantization (floor(.*PV)/PV, <=1.5e-6) and the conv
    bias (biases/PV2 ~ 1e-6) are dropped; both are inside the fit anyway.
  * Layout: image rows in 16 blocks of 16; partitions hold (block, channel).
    M=16 lets a chain pack a block OCTET: h tiles are [128=(8 blocks, 16
    units), 1024 px], so only 32 chains (ct, f, octet) cover the image.
  * Conv: 25 accumulating fp32r matmuls per 512-px subtile (K=128, M=64);
    ct0 runs up front, ct1..3 pop into the pipeline's spare PE slots.
  * Chains run in a 2-step-skewed pipeline: step k issues basis matmuls of
    chain k, the biased tanh of chain k-1, and the A-projection matmuls of
    chain k-2, so the Act engine never waits on same-chain latency.
  * The A-projection writes filter-major [128=(blk,c), 512] psum tiles in
    the x layout (zero-padded M=128, octet bands accumulated); the cross-
    filter 3rd-largest runs as a running top-3 insertion on the vector
    engine, full width, straight from PSUM.
  * The final tanh commutes with the 3rd-largest selection (monotone):
    insert network on pre-tanh values, then one 1024-wide tanh and
    clip(x + update_rate*z3) per column tile.
  PSUM: chain ring 2x[128,1024] (4 banks) + T ring 2x[128,512] (1+1) +
  conv accumulator [64,1024] (2) = 8 banks.
"""

import os
import numpy as np

import concourse.bass as bass
import concourse.bacc as bacc
import concourse.mybir as mybir
from concourse.tile import TileContext
from concourse.bass_utils import run_bass_kernel_spmd

F32 = mybir.dt.float32
F32R = mybir.dt.float32r
AF = mybir.ActivationFunctionType
ALU = mybir.AluOpType

B, C, H, W = 8, 8, 256, 256
F = 4
RK, HALO = 5, 2
PV1 = float(np.floor(2**31 / 128))
PV2 = float(np.floor(2**31 / (RK * RK * 128)))

NBLK, RB = 16, 16          # 16 row-blocks of 16 rows
ROWS, COLS = RB + 2 * HALO, W + 2 * HALO      # 20, 260
NPIX = RB * W                                 # 4096 pixels per block
CT = 4                                        # column tiles of 1024
CTW = NPIX // CT                              # 1024
SUB = 512                                     # matmul moving-dim tile
AROWS, BROW0, BROWS = 8, 6, 14                # x band split (frame rows)
AFREE, BFREE = AROWS * COLS, BROWS * COLS     # 2080, 3640
NCH = CT * F * 2                              # 32 chains, (ct, f, octet)
M = 16                                        # tanh basis size

_cache = {}
LAST_RESULTS = None


def _totalistic(k):
    def sym(a):
        return a + np.flip(a, -2) + np.flip(a, -1) + np.flip(a, (-2, -1))
    z = 0.125 * (sym(k) + sym(np.swapaxes(k, -2, -1)))
    return z - z.mean(axis=(-2, -1), keepdims=True)


def _fit_basis(x, kt, W1, W2, W3, W4):
    """Fit u(p) ~ A @ tanh(beta*p + gamma) per filter over the actual p
    range of this input.  Returns beta [F,M], gamma [F,M], A [F,M,8]."""
    z = np.floor(x.astype(np.float32) * PV2) / PV2

    def G(f, p):
        h = np.tanh(W1[f][:, 0:1].astype(np.float64) * p[None, :])
        h = np.tanh(W2[f].astype(np.float64) @ h)
        h = np.tanh(W3[f].astype(np.float64) @ h)
        return (W4[f].astype(np.float64) @ h).T          # [N, 8]

    betas = np.zeros((F, M), np.float64)
    gammas = np.zeros((F, M), np.float64)
    As = np.zeros((F, M, 8), np.float64)
    for f in range(F):
        acc = np.zeros_like(z[:, 0])                     # [B, H, W]
        for c in range(C):
            for dy in range(RK):
                for dx in range(RK):
                    acc += kt[f, c, dy, dx] * np.roll(
                        z[:, c], (HALO - dy, HALO - dx), (-2, -1))
        lo, hi = float(acc.min()) - 0.3, float(acc.max()) + 0.3
        flo = float(kt[f][kt[f] < 0].sum()) - 1.0
        fhi = float(kt[f][kt[f] > 0].sum()) + 1.0

        grid = np.linspace(lo, hi, 4000)
        tail = np.concatenate([np.linspace(flo, lo, 200)[:-5],
                               np.linspace(hi, fhi, 200)[5:]])
        allp = np.concatenate([grid, tail])
        w = np.concatenate([np.ones(len(grid)), 0.05 * np.ones(len(tail))])
        U = G(f, allp)

        dU = np.abs(np.diff(G(f, grid), axis=0)).sum(1)
        cdf = np.cumsum(dU)
        cdf /= cdf[-1]
        centers = np.interp((np.arange(M) + 0.5) / M, cdf, grid[1:])
        sp = np.gradient(np.sort(centers))
        beta = 1.0 / np.clip(sp, 0.05, 10.0)
        gamma = -beta * centers
        theta = np.concatenate([beta, gamma])

        def model(theta):
            b, g = theta[:M], theta[M:]
            Phi = np.tanh(b[None, :] * allp[:, None] + g[None, :])
            A = np.linalg.lstsq(Phi * w[:, None], U * w[:, None], rcond=None)[0]
            return Phi, A, Phi @ A - U

        lam = 1e-2
        Phi, A, R = model(theta)
        cost = (w[:, None] * R ** 2).sum()
        wf = np.repeat(w, 8)
        for _ in range(60):
            S = 1 - Phi ** 2
            J = np.empty((len(allp) * 8, 2 * M))
            PS = allp[:, None] * S
            for m in range(M):
                J[:, m] = np.outer(PS[:, m], A[m]).ravel()
                J[:, M + m] = np.outer(S[:, m], A[m]).ravel()
            JTJ = (J * wf[:, None]).T @ J
            JTr = (J * wf[:, None]).T @ R.ravel()
            improved = False
            for _tries in range(8):
                try:
                    step = np.linalg.solve(
                        JTJ + lam * np.diag(np.diag(JTJ) + 1e-9), JTr)
                except np.linalg.LinAlgError:
                    lam *= 10
                    continue
                t2 = theta - step
                Phi2, A2, R2 = model(t2)
                c2 = (w[:, None] * R2 ** 2).sum()
                if c2 < cost:
                    theta, Phi, A, R, cost = t2, Phi2, A2, R2, c2
                    lam = max(lam * 0.5, 1e-6)
                    improved = True
                    break
                lam *= 4
            if not improved:
                break
        betas[f], gammas[f], As[f] = theta[:M], theta[M:], A
    return betas, gammas, As


def _prep_weights(x, kernels, biases, W1, W2, W3, W4):
    kt = _totalistic(kernels.astype(np.float64)).astype(np.float32)  # [F,C,5,5]
    beta, gamma, A = _fit_basis(x, kt.astype(np.float64), W1, W2, W3, W4)

    # conv lhsT: [128=(blk,c), 25*64]; col tap*64 + (f*16+blk)
    convw = np.zeros((128, 25 * 64), np.float32)
    for t in range(25):
        dy, dx = divmod(t, 5)
        for blk in range(NBLK):
            for c in range(C):
                for f in range(F):
                    convw[blk * 8 + c, t * 64 + f * 16 + blk] = kt[f, c, dy, dx]

    # basis lhsT: [64=(f,blk), 8*128]; col (f*2+o)*128 + (b8*16+m) = beta
    l1w = np.zeros((64, 8 * 128), np.float32)
    for f in range(F):
        for o in range(2):
            for b8 in range(8):
                l1w[f * 16 + o * 8 + b8,
                    (f * 2 + o) * 128 + b8 * 16:(f * 2 + o) * 128 + b8 * 16 + M] = beta[f]

    # gamma bias tile [128=(b8,m), F]
    gam = np.zeros((128, F), np.float32)
    for f in range(F):
        for b8 in range(8):
            gam[b8 * 16:b8 * 16 + M, f] = gamma[f]

    # A-projection lhsT: per (o, f) a zero-padded [128, 128] block writing
    # the octet's 64-row band of the x-layout tile T_f
    l4w = np.zeros((128, 8 * 128), np.float32)
    for o in range(2):
        for f in range(F):
            base = (o * 4 + f) * 128
            for b8 in range(8):
                for cc in range(C):
                    l4w[b8 * 16:b8 * 16 + M,
                        base + o * 64 + b8 * 8 + cc] = A[f][:, cc]
    return convw, l1w, gam, l4w


def _stage_x(xb):
    """xb: [C, H, W] -> [128=(blk,c), AFREE+BFREE]: circular-halo frame rows
    0..8 then rows 6..20."""
    frame = np.empty((128, ROWS, COLS), np.float32)
    rows = (np.arange(-HALO, RB + HALO)[None, :] + np.arange(NBLK)[:, None] * RB) % H
    cols = np.arange(-HALO, W + HALO) % W
    for blk in range(NBLK):
        frame[blk * 8:blk * 8 + 8] = xb[:, rows[blk]][:, :, cols]
    out = np.concatenate(
        [frame[:, :AROWS].reshape(128, AFREE),
         frame[:, BROW0:BROW0 + BROWS].reshape(128, BFREE)], axis=1)
    return np.ascontiguousarray(out)


def _build_nc(update_rate):
    nc = bacc.Bacc(trn_type="TRN2")

    xd = nc.dram_tensor("xsb", [128, AFREE + BFREE], F32R, kind="ExternalInput")
    cwd = nc.dram_tensor("convw", [128, 1600], F32R, kind="ExternalInput")
    w1d = nc.dram_tensor("l1w", [64, 8 * 128], F32R, kind="ExternalInput")
    gmd = nc.dram_tensor("gam", [128, F], F32, kind="ExternalInput")
    w4d = nc.dram_tensor("l4w", [128, 8 * 128], F32R, kind="ExternalInput")
    outd = nc.dram_tensor("out", [128, NPIX], F32, kind="ExternalOutput")

    ur = float(update_rate)

    with TileContext(nc) as tc:
        with (
            tc.tile_pool(name="w", bufs=1) as wp,
            tc.tile_pool(name="sb", bufs=2) as sp,
            tc.tile_pool(name="hh", bufs=6) as hp,
            tc.tile_pool(name="psc", bufs=2, space="PSUM") as cp,
            tc.tile_pool(name="pst", bufs=3, space="PSUM") as tp,
            tc.tile_pool(name="psv", bufs=1, space="PSUM") as cv,
        ):
            xa = wp.tile([128, AFREE], F32R, tag="xa")
            xbt = wp.tile([128, BFREE], F32R, tag="xb")
            cw = wp.tile([128, 1600], F32R, tag="cw")
            w1 = wp.tile([64, 8 * 128], F32R, tag="w1")
            gm = wp.tile([128, F], F32, tag="gm")
            w4 = wp.tile([128, 8 * 128], F32R, tag="w4")
            p_sbs = [wp.tile([64, CTW], F32R, tag=f"p{i}", name=f"p_{i}")
                     for i in range(CT)]
            out_sbs = [wp.tile([128, CTW], F32, tag=f"o{i}", name=f"o_{i}")
                       for i in range(CT)]

            nc.sync.dma_start(out=xa[:], in_=xd[:, 0:AFREE])
            nc.sync.dma_start(out=cw[:, 0:832], in_=cwd[:, 0:832])
            nc.sync.dma_start(out=cw[:, 832:1600], in_=cwd[:, 832:1600])
            nc.sync.dma_start(out=xbt[:], in_=xd[:, AFREE:AFREE + BFREE])
            nc.sync.dma_start(out=w1[:], in_=w1d[:])
            nc.sync.dma_start(out=gm[:], in_=gmd[:])
            nc.sync.dma_start(out=w4[:], in_=w4d[:])

            xra = xa[:].rearrange("p (r c) -> p r c", c=COLS)   # rows 0..8
            xrb = xbt[:].rearrange("p (r c) -> p r c", c=COLS)  # rows 6..20

            conv_acc = {}

            def conv_items(ct):
                for s in range(2):
                    for t in range(25):
                        def mm(t=t, s=s, ct=ct):
                            if t == 0:
                                conv_acc[(ct, s)] = cv.tile(
                                    [64, SUB], F32, tag="v",
                                    name=f"cv_{ct}_{s}")
                            dy, dx = divmod(t, 5)
                            r0 = 4 * ct + 2 * s + dy
                            if r0 >= BROW0:
                                rhs = xrb[:, r0 - BROW0:r0 - BROW0 + 2,
                                          dx:dx + W]
                            else:
                                rhs = xra[:, r0:r0 + 2, dx:dx + W]
                            outap = conv_acc[(ct, s)][0:64, :] \
                                .rearrange("p (a b) -> p a b", b=W)
                            nc.tensor.matmul(
                                outap, lhsT=cw[:, t * 64:t * 64 + 64],
                                rhs=rhs, start=(t == 0), stop=(t == 24))
                        yield mm

                    def pcopy(ct=ct, s=s):
                        nc.vector.tensor_copy(
                            p_sbs[ct][:, s * SUB:(s + 1) * SUB],
                            conv_acc[(ct, s)][0:64, :])
                    yield pcopy

            for it in conv_items(0):
                it()

            # ---- 2-step-skewed chain pipeline over 32 chains ----
            # chain j = (ct=j//8, f=(j//2)%4, o=j%2)
            wq = []        # conv filler
            mreg = {}      # (ct, s, m) running top-3 tiles
            z3cts = {}     # ct -> [128, CTW] pre-output tile
            ht = [None] * NCH
            tf = {}        # (f, s) -> live psum tile
            pend_s1 = []   # deferred s=1 inserts from the finished group

            def pop_work(n):
                for _ in range(n):
                    if wq:
                        wq.pop(0)()

            def insert_ops(fi, T, s, ct):
                """Running top-3 insert of T (=[128,SUB] psum) for half s."""
                if fi == 0:
                    for m in range(3):
                        mreg[(ct, s, m)] = sp.tile(
                            [128, SUB], F32, tag=f"m{m}{s}",
                            name=f"m{m}_{ct}_{s}")
                m1, m2, m3 = (mreg[(ct, s, m)] for m in range(3))
                if fi == 0:
                    nc.scalar.copy(m1[:], T[:])
                elif fi == 1:
                    nc.vector.tensor_tensor(m2[:], m1[:], T[:], ALU.min)
                    nc.vector.tensor_tensor(m1[:], m1[:], T[:], ALU.max)
                elif fi == 2:
                    lo = sp.tile([128, SUB], F32, tag="tt",
                                 name=f"tt_{ct}_{s}")
                    nc.vector.tensor_tensor(lo[:], m1[:], T[:], ALU.min)
                    nc.vector.tensor_tensor(m3[:], m2[:], lo[:], ALU.min)
                    nc.vector.tensor_tensor(m2[:], m2[:], lo[:], ALU.max)
                else:
                    # z3 half = 3rd largest = max(m3, min(m2, T))
                    if ct not in z3cts:
                        z3cts[ct] = sp.tile([128, CTW], F32, tag="z3",
                                            name=f"z3_{ct}")
                    zs = z3cts[ct][:, s * SUB:(s + 1) * SUB]
                    nc.vector.tensor_tensor(zs, m2[:], T[:], ALU.min)
                    nc.vector.tensor_tensor(zs, m3[:], zs, ALU.max)
                    if s == 1:
                        # full-tile tanh + output
                        z3 = z3cts[ct]
                        nc.scalar.activation(z3[:], z3[:], AF.Tanh)
                        if ur != 1.0:
                            nc.vector.tensor_scalar_mul(z3[:], z3[:], ur)
                        r = 4 * ct
                        if ct == 0:
                            xv = xra[:, HALO + r:HALO + r + 4, HALO:HALO + W]
                        else:
                            xv = xrb[:, HALO + r - BROW0:HALO + r - BROW0 + 4,
                                     HALO:HALO + W]
                        ov = out_sbs[ct][:].rearrange("p (a b) -> p a b", b=W)
                        nc.vector.tensor_tensor(
                            ov, xv.bitcast(F32),
                            z3[:].rearrange("p (a b) -> p a b", b=W), ALU.add)
                        nc.vector.tensor_scalar(
                            out_sbs[ct][:], out_sbs[ct][:],
                            0.0, 1.0, ALU.max, ALU.min)
                        nc.sync.dma_start(
                            out=outd[:, ct * CTW:(ct + 1) * CTW],
                            in_=out_sbs[ct][:])

            ch_hist = {}
            for step in range(NCH + 2):
                if step % 8 == 0 and step // 8 + 1 < CT:
                    wq.extend(conv_items(step // 8 + 1))
                pop_work(6)
                if step < NCH:
                    j, ct = step, step // 8
                    f, o = (j // 2) % 4, j % 2
                    ch = cp.tile([128, CTW], F32, tag="c", name=f"ch_{j}")
                    for s in range(2):
                        nc.tensor.matmul(
                            ch[:, s * SUB:(s + 1) * SUB],
                            lhsT=w1[:, (f * 2 + o) * 128:(f * 2 + o + 1) * 128],
                            rhs=p_sbs[ct][0:64, s * SUB:(s + 1) * SUB],
                            start=True, stop=True)
                    ch_hist[j] = ch
                if 1 <= step < NCH + 1:
                    j = step - 1
                    ct, f = j // 8, (j // 2) % 4
                    ht[j] = hp.tile([128, CTW], F32R, tag="h", name=f"h_{j}")
                    nc.scalar.activation(ht[j][:], ch_hist.pop(j)[:], AF.Tanh,
                                         bias=gm[:, f:f + 1])

                while pend_s1:
                    pend_s1.pop(0)()

                pop_work(4)
                if 2 <= step:
                    j = step - 2
                    ct, f, o = j // 8, (j // 2) % 4, j % 2
                    if o == 0:
                        tf[(f, 0)] = tp.tile([128, SUB], F32, tag="t",
                                             name=f"tf0_{ct}_{f}")
                        tf[(f, 1)] = tp.tile([128, SUB], F32, tag="t",
                                             name=f"tf1_{ct}_{f}")
                    for s in range(2):
                        nc.tensor.matmul(
                            tf[(f, s)][:, :],
                            lhsT=w4[:, (o * 4 + f) * 128:(o * 4 + f + 1) * 128],
                            rhs=ht[j][:, s * SUB:(s + 1) * SUB],
                            start=(o == 0), stop=(o == 1))
                    if o == 1:
                        insert_ops(f, tf[(f, 0)], 0, ct)
                        T1 = tf[(f, 1)]

                        def s1b(f=f, ct=ct, T1=T1):
                            insert_ops(f, T1, 1, ct)
                        pend_s1.append(s1b)

            while wq or pend_s1:
                while pend_s1:
                    pend_s1.pop(0)()
                pop_work(1)
    nc.finalize()
    return nc


def kernel(x, kernels, biases, W1, W2, W3, W4, update_rate):
    global LAST_RESULTS
    x = np.ascontiguousarray(np.asarray(x, dtype=np.float32))
    kernels = np.asarray(kernels, dtype=np.float32)
    biases = np.asarray(biases, dtype=np.float32)
    W1 = np.asarray(W1, dtype=np.float32)
    W2 = np.asarray(W2, dtype=np.float32)
    W3 = np.asarray(W3, dtype=np.float32)
    W4 = np.asarray(W4, dtype=np.float32)
    ur = float(np.asarray(update_rate))

    key = ("nc", ur)
    if key not in _cache:
        _cache[key] = _build_nc(ur)
    nc = _cache[key]

    convw, l1w, gam, l4w = _prep_weights(
        x, kernels, biases, W1, W2, W3, W4)
    shared = {
        "convw": np.ascontiguousarray(convw),
        "l1w": np.ascontiguousarray(l1w),
        "gam": np.ascontiguousarray(gam),
        "l4w": np.ascontiguousarray(l4w),
    }
    in_maps = []
    for b in range(B):
        m = dict(shared)
        m["xsb"] = _stage_x(x[b])
        in_maps.append(m)

    trace = bool(int(os.environ.get("KERNEL_TRACE", "0")))
    res = run_bass_kernel_spmd(nc, in_maps, list(range(B)), trace=trace)
    LAST_RESULTS = res

    out = np.empty((B, C, H, W), np.float32)
    for b in range(B):
        ob = res.results[b]["out"].reshape(NBLK, C, RB, W)
        out[b] = ob.transpose(1, 0, 2, 3).reshape(C, H, W)
    return out
